# revision 11
# baseline (speedup 1.0000x reference)
"""Trainium2 Bass kernel for a bidirectional GRU encoder (NMT style).

Problem: V=32000, E=512, H=1024, B=64, S=256.
  x_emb = emb[x_source]                        (B, S, E)
  h_fwd = GRU_f(x_emb)                         (B, S, H)
  h_rev = realign(GRU_r(realign(x_emb)))       (B, S, H)
  hidd_out = concat([h_fwd, h_rev], -1)        (B, S, 2H)
  seq_hidd = hidd_out[b, len_b-1]              (B, 2H)

Sharding (8 NeuronCores): core c owns h-dims [128c, 128(c+1)) of BOTH
directions.  SBUF layout is batch-on-partitions: partitions 0:64 carry the
fwd batch, 64:128 the rev batch; h-dims/gates live on the free axis.

Per step the recurrent matmul streams this core's W_hh.T slice (1024, 384)
through the PE with h.T K-tiles stationary; the full h.T (1024, 64+64) is
reassembled every step with an 8-core AllGather of each core's transposed
h_new slice.  b_hh(r,z) and b_ih fold into the precomputed x-projection;
b_hh(n) is added via a K=1 ones-row matmul inside the scan.

The embedding lookup and the per-sample time realign of the reverse input
stream happen host-side (this runtime's dynamic-DGE gather ops crash the
worker): each core receives its 1/8 chunk of BOTH token streams already
transposed to (E, token).  The reverse outputs are realigned back on the
host during assembly.  All matmuls, the full 256-step recurrence, and all
gate math run on device.  The bass program is input-value independent, so
it compiles once per process.

Phases:
  P2 two AllGathers of the per-core (E, token) chunks (fwd + rev streams)
  P3 GEMM: xp_{f,r} = emb_tok @ W_ih(slice).T + biases, t-major rows
  P5 256-step scan (both directions at once)
"""
import os
import sys

if "/opt/trn_rl_repo" not in sys.path:
    sys.path.insert(0, "/opt/trn_rl_repo")

import numpy as np

V, E, H, B, S = 32000, 512, 1024, 64, 256
NC_ = 8          # cores
HS = H // NC_    # 128 h-dims per core
GS = 3 * HS      # 384 gate rows per direction per core
TOK = S * B      # 16384 tokens, t-major (t*B + b)
TPC = TOK // NC_  # 2048 tokens per core chunk
ET = E // 128    # 4 embedding K-tiles
KT = H // 128    # 8 h K-tiles

_BUILT = None
# test-only overrides
_NSTEPS = int(os.environ.get("KERNEL_NSTEPS", str(S)))
_PHASES = set(os.environ.get("KERNEL_PHASES", "235"))


def _build_bass():
    """Build + compile the (input-independent) 8-core SPMD bass program."""
    import concourse.bass as bass  # noqa: F401
    import concourse.bacc as bacc
    import concourse.mybir as mybir
    import concourse.tile as tile
    from concourse.masks import make_identity

    f32 = mybir.dt.float32
    ADD = mybir.AluOpType.add
    SUB = mybir.AluOpType.subtract
    MUL = mybir.AluOpType.mult
    SIG = mybir.ActivationFunctionType.Sigmoid
    TANH = mybir.ActivationFunctionType.Tanh

    nc = bacc.Bacc(None, target_bir_lowering=False)

    # ---------------- I/O ----------------
    embtf = nc.dram_tensor("embtf", [ET, 128, TPC], f32, kind="ExternalInput")
    embtr = nc.dram_tensor("embtr", [ET, 128, TPC], f32, kind="ExternalInput")
    widh = nc.dram_tensor("widh", [ET, 128, 2 * GS], f32, kind="ExternalInput")
    biasc = nc.dram_tensor("biasc", [1, 2 * GS], f32, kind="ExternalInput")
    whf = nc.dram_tensor("whf", [KT, 128, GS], f32, kind="ExternalInput")
    whr = nc.dram_tensor("whr", [KT, 128, GS], f32, kind="ExternalInput")
    bhnf = nc.dram_tensor("bhnf", [1, GS], f32, kind="ExternalInput")
    bhnr = nc.dram_tensor("bhnr", [1, GS], f32, kind="ExternalInput")
    ones_in = nc.dram_tensor("ones_in", [1, 128], f32, kind="ExternalInput")

    out_f = nc.dram_tensor("out_f", [TOK, HS], f32, kind="ExternalOutput")
    out_r = nc.dram_tensor("out_r", [TOK, HS], f32, kind="ExternalOutput")

    with tile.TileContext(nc) as tc:
        with (
            tc.tile_pool(name="const", bufs=1) as constp,
            tc.tile_pool(name="psg", bufs=4, space="PSUM") as psg,
            tc.tile_pool(name="ptr", bufs=2, space="PSUM") as ptr,
            tc.tile_pool(name="dram", bufs=1, space="DRAM") as dram,
            tc.tile_pool(name="dbounce", bufs=2, space="DRAM") as dbounce,
        ):
            # ---------------- constants / weights to SBUF ----------------
            ident = constp.tile([128, 128], f32)
            make_identity(nc, ident[:])

            widh_sb = constp.tile([128, ET, 2 * GS], f32)
            nc.sync.dma_start(widh_sb[:], widh[:].rearrange("e p f -> p e f"))
            biasc_sb = constp.tile([1, 2 * GS], f32)
            nc.sync.dma_start(biasc_sb[:], biasc[:])
            whf_sb = constp.tile([128, KT, GS], f32)
            nc.sync.dma_start(whf_sb[:], whf[:].rearrange("k p f -> p k f"))
            whr_sb = constp.tile([128, KT, GS], f32)
            nc.sync.dma_start(whr_sb[:], whr[:].rearrange("k p f -> p k f"))
            bhnf_sb = constp.tile([1, GS], f32)
            nc.sync.dma_start(bhnf_sb[:], bhnf[:])
            bhnr_sb = constp.tile([1, GS], f32)
            nc.sync.dma_start(bhnr_sb[:], bhnr[:])
            ones_sb = constp.tile([1, 128], f32)
            nc.sync.dma_start(ones_sb[:], ones_in[:])

            embf_my = dram.tile([ET, 128, TPC], f32)
            embr_my = dram.tile([ET, 128, TPC], f32)
            embf_full = dram.tile(
                [NC_, ET, 128, TPC], f32, addr_space="Shared"
            )
            embr_full = dram.tile(
                [NC_, ET, 128, TPC], f32, addr_space="Shared"
            )
            xp_f = dram.tile([TOK, GS], f32)   # 25 MB
            xp_r = dram.tile([TOK, GS], f32)

            # ---------------- P2: AllGather embedding streams ------------
            if "2" in _PHASES:
                nc.gpsimd.dma_start(embf_my[:], embtf[:])
                nc.gpsimd.dma_start(embr_my[:], embtr[:])
                nc.gpsimd.collective_compute(
                    "AllGather",
                    mybir.AluOpType.bypass,
                    ins=[embf_my.opt()],
                    outs=[embf_full.opt()],
                    replica_groups=[list(range(NC_))],
                )
                nc.gpsimd.collective_compute(
                    "AllGather",
                    mybir.AluOpType.bypass,
                    ins=[embr_my.opt()],
                    outs=[embr_full.opt()],
                    replica_groups=[list(range(NC_))],
                )

            # ---------------- P3: x-projection GEMM ----------------
            with tc.tile_pool(name="p3", bufs=3) as p3:
                n_tt = (TOK // 128) if "3" in _PHASES else 0
                for i in range(n_tt):
                    r_, j = divmod(i, TPC // 128)
                    lf = p3.tile([128, ET, 128], f32, tag="lf")
                    nc.sync.dma_start(
                        lf[:],
                        embf_full[r_:r_ + 1, :, :, j * 128:(j + 1) * 128]
                        .rearrange("r e p t -> p (r e) t"),
                    )
                    lr = p3.tile([128, ET, 128], f32, tag="lr")
                    nc.sync.dma_start(
                        lr[:],
                        embr_full[r_:r_ + 1, :, :, j * 128:(j + 1) * 128]
                        .rearrange("r e p t -> p (r e) t"),
                    )
                    pf = psg.tile([128, GS], f32, tag="g")
                    pr = psg.tile([128, GS], f32, tag="g")
                    for e in range(ET):
                        nc.tensor.matmul(
                            pf[:], lf[:, e, :], widh_sb[:, e, 0:GS],
                            start=(e == 0), stop=False,
                        )
                    nc.tensor.matmul(
                        pf[:], ones_sb[0:1, 0:128], biasc_sb[0:1, 0:GS],
                        start=False, stop=True,
                    )
                    for e in range(ET):
                        nc.tensor.matmul(
                            pr[:], lr[:, e, :], widh_sb[:, e, GS:2 * GS],
                            start=(e == 0), stop=False,
                        )
                    nc.tensor.matmul(
                        pr[:], ones_sb[0:1, 0:128], biasc_sb[0:1, GS:2 * GS],
                        start=False, stop=True,
                    )
                    xf_sb = p3.tile([128, GS], f32, tag="xf")
                    xr_sb = p3.tile([128, GS], f32, tag="xr")
                    nc.vector.tensor_copy(xf_sb[:], pf[:])
                    nc.vector.tensor_copy(xr_sb[:], pr[:])
                    nc.sync.dma_start(xp_f[i * 128:(i + 1) * 128, :], xf_sb[:])
                    nc.sync.dma_start(xp_r[i * 128:(i + 1) * 128, :], xr_sb[:])

            # ---------------- P5: the scan ----------------
            with (
                tc.tile_pool(name="sc2", bufs=2) as sc2,
                tc.tile_pool(name="sc3", bufs=3) as sc3,
            ):
                hT_prev = sc2.tile([128, KT, 128], f32, tag="hT0")
                nc.vector.memzero(hT_prev[:])
                hprev = sc2.tile([128, HS], f32, tag="hp0")
                nc.vector.memzero(hprev[:])

                for t in range(_NSTEPS if "5" in _PHASES else 0):
                    # gates = [h.T @ whf ; h.T @ whr] + bhn (ones-row mm)
                    pg = psg.tile([128, GS], f32, tag="g")
                    for k in range(KT):
                        nc.tensor.matmul(
                            pg[0:64, :], hT_prev[:, k, 0:64], whf_sb[:, k, :],
                            start=(k == 0), stop=False,
                        )
                    nc.tensor.matmul(
                        pg[0:64, :], ones_sb[0:1, 0:64], bhnf_sb[0:1, :],
                        start=False, stop=True,
                    )
                    for k in range(KT):
                        nc.tensor.matmul(
                            pg[64:128, :], hT_prev[:, k, 64:128],
                            whr_sb[:, k, :],
                            start=(k == 0), stop=False,
                        )
                    nc.tensor.matmul(
                        pg[64:128, :], ones_sb[0:1, 0:64], bhnr_sb[0:1, :],
                        start=False, stop=True,
                    )

                    # x-projection rows for this step
                    xpt = sc3.tile([128, GS], f32, tag="xpt")
                    nc.sync.dma_start(xpt[0:64, :], xp_f[t * B:(t + 1) * B, :])
                    nc.sync.dma_start(
                        xpt[64:128, :], xp_r[t * B:(t + 1) * B, :]
                    )

                    # gate math; free-axis blocks: [r | z | n] x 128
                    arz = sc3.tile([128, 2 * HS], f32, tag="arz")
                    nc.vector.tensor_tensor(
                        arz[:], pg[:, 0:2 * HS], xpt[:, 0:2 * HS], ADD
                    )
                    rz = sc3.tile([128, 2 * HS], f32, tag="rz")
                    nc.scalar.activation(rz[:], arz[:], SIG)
                    t1 = sc3.tile([128, HS], f32, tag="t1")
                    nc.vector.tensor_tensor(
                        t1[:], rz[:, 0:HS], pg[:, 2 * HS:3 * HS], MUL
                    )
                    t2 = sc3.tile([128, HS], f32, tag="t2")
                    nc.vector.tensor_tensor(
                        t2[:], t1[:], xpt[:, 2 * HS:3 * HS], ADD
                    )
                    n_t = sc3.tile([128, HS], f32, tag="nt")
                    nc.scalar.activation(n_t[:], t2[:], TANH)
                    d_t = sc3.tile([128, HS], f32, tag="dt")
                    nc.vector.tensor_tensor(d_t[:], hprev[:], n_t[:], SUB)
                    zd = sc3.tile([128, HS], f32, tag="zd")
                    nc.vector.tensor_tensor(zd[:], rz[:, HS:2 * HS], d_t[:], MUL)
                    hnew = sc3.tile([128, HS], f32, tag="hnew")
                    nc.vector.tensor_tensor(hnew[:], n_t[:], zd[:], ADD)

                    # store outputs (t-major rows; rev realigned on host)
                    nc.sync.dma_start(
                        out_f[t * B:(t + 1) * B, :], hnew[0:64, :]
                    )
                    nc.sync.dma_start(
                        out_r[t * B:(t + 1) * B, :], hnew[64:128, :]
                    )

                    # transpose own slice and exchange
                    pt = ptr.tile([128, 128], f32, tag="ptr")
                    nc.tensor.transpose(pt[:], hnew[:], ident[:])
                    snd = sc3.tile([128, 128], f32, tag="snd")
                    nc.vector.tensor_copy(snd[:], pt[:])
                    bounce_in = dbounce.tile([128, 128], f32, tag="bin")
                    bounce_out = dbounce.tile(
                        [NC_, 128, 128], f32, tag="bout", addr_space="Shared"
                    )
                    nc.gpsimd.dma_start(bounce_in[:], snd[:])
                    nc.gpsimd.collective_compute(
                        "AllGather",
                        mybir.AluOpType.bypass,
                        ins=[bounce_in.opt()],
                        outs=[bounce_out.opt()],
                        replica_groups=[list(range(NC_))],
                    )
                    hT_new = sc2.tile([128, KT, 128], f32, tag="hTn")
                    nc.sync.dma_start(
                        hT_new[:], bounce_out[:].rearrange("k p f -> p k f")
                    )
                    hT_prev = hT_new
                    hprev = hnew

    nc.compile()
    return nc


def _get_built():
    global _BUILT
    if _BUILT is None:
        _BUILT = _build_bass()
    return _BUILT


def prep_in_maps(inputs):
    x_source = np.asarray(inputs["x_source"])
    x_lengths = np.asarray(inputs["x_lengths"])
    emb = np.asarray(inputs["emb"], dtype=np.float32)
    w_ih_f = np.asarray(inputs["w_ih_f"], np.float32)
    w_hh_f = np.asarray(inputs["w_hh_f"], np.float32)
    b_ih_f = np.asarray(inputs["b_ih_f"], np.float32)
    b_hh_f = np.asarray(inputs["b_hh_f"], np.float32)
    w_ih_r = np.asarray(inputs["w_ih_r"], np.float32)
    w_hh_r = np.asarray(inputs["w_hh_r"], np.float32)
    b_ih_r = np.asarray(inputs["b_ih_r"], np.float32)
    b_hh_r = np.asarray(inputs["b_hh_r"], np.float32)

    lengths = x_lengths.astype(np.int64)

    # realign index map: idx[b, t] = S-1-((t + S-len_b) % S)
    t_ = np.arange(S)[None, :]
    idx = S - 1 - ((t_ + (S - lengths)[:, None]) % S)        # (B, S)

    tok_f = x_source.T.astype(np.int64)                      # (S, B) t-major
    tok_r = np.take_along_axis(x_source, idx, axis=1).T      # (S, B) realigned

    # (E, token) streams, chunked per core: (NC, ET, 128, TPC)
    def chunks(tok):
        xe = emb[tok.reshape(-1)]                            # (TOK, E)
        xeT = np.ascontiguousarray(xe.T)                     # (E, TOK)
        return np.ascontiguousarray(
            xeT.reshape(ET, 128, NC_, TPC).transpose(2, 0, 1, 3)
        )

    embf_c = chunks(tok_f)
    embr_c = chunks(tok_r)

    in_maps = []
    for c in range(NC_):
        rows = np.concatenate(
            [np.arange(c * HS, (c + 1) * HS) + off * H for off in range(3)]
        )
        zeros = np.zeros(HS, np.float32)
        bias_f = b_ih_f[rows] + np.concatenate(
            [b_hh_f[rows[:HS]], b_hh_f[rows[HS:2 * HS]], zeros]
        )
        bias_r = b_ih_r[rows] + np.concatenate(
            [b_hh_r[rows[:HS]], b_hh_r[rows[HS:2 * HS]], zeros]
        )
        widh_np = np.concatenate(
            [w_ih_f[rows, :].T, w_ih_r[rows, :].T], axis=1
        ).reshape(ET, 128, 2 * GS)
        whf_np = np.ascontiguousarray(w_hh_f[rows, :].T).reshape(KT, 128, GS)
        whr_np = np.ascontiguousarray(w_hh_r[rows, :].T).reshape(KT, 128, GS)
        bhn_f = np.concatenate([zeros, zeros, b_hh_f[rows[2 * HS:]]])
        bhn_r = np.concatenate([zeros, zeros, b_hh_r[rows[2 * HS:]]])

        in_maps.append({
            "embtf": embf_c[c],
            "embtr": embr_c[c],
            "widh": np.ascontiguousarray(widh_np, np.float32),
            "biasc": np.concatenate([bias_f, bias_r]).reshape(1, -1)
                       .astype(np.float32),
            "whf": np.ascontiguousarray(whf_np, np.float32),
            "whr": np.ascontiguousarray(whr_np, np.float32),
            "bhnf": bhn_f.reshape(1, -1).astype(np.float32),
            "bhnr": bhn_r.reshape(1, -1).astype(np.float32),
            "ones_in": np.ones((1, 128), np.float32),
        })
    return in_maps, (lengths, idx)


def assemble(results, meta):
    lengths, idx = meta
    hidd = np.empty((B, S, 2 * H), np.float32)
    out_r_all = np.empty((B, S, H), np.float32)
    for c in range(NC_):
        of = results[c]["out_f"].reshape(S, B, HS)
        orr = results[c]["out_r"].reshape(S, B, HS)
        hidd[:, :, c * HS:(c + 1) * HS] = of.transpose(1, 0, 2)
        out_r_all[:, :, c * HS:(c + 1) * HS] = orr.transpose(1, 0, 2)
    # realign reverse outputs back into natural time order (host)
    hidd[:, :, H:] = np.take_along_axis(out_r_all, idx[:, :, None], axis=1)
    seq = hidd[np.arange(B), lengths - 1]
    return hidd, seq


def kernel(**inputs):
    in_maps, meta = prep_in_maps(inputs)
    nc = _get_built()
    from concourse.bass_utils import run_bass_kernel_spmd

    last_err = None
    for _attempt in range(3):
        try:
            res = run_bass_kernel_spmd(
                nc, in_maps, core_ids=list(range(NC_)), trace=False
            )
            return assemble(res.results, meta)
        except Exception as e:  # transient device wedges recover on retry
            last_err = e
    raise last_err
